# revision 1
# baseline (speedup 1.0000x reference)
"""DFlashAttention on 8 Trainium2 NeuronCores.

Sharding: data-parallel over batch (4) x tensor-parallel over heads (2).
Core c = 2*b + t handles batch b, q heads [16t, 16t+16), kv heads [4t, 4t+4).
GQA groups (4 q heads per kv head) align with the head split, so attention is
core-local. Output projection: AllGather of attention outputs over the TP
pair, then each core computes its [128, 2048] output column block.

Fused single-pass pipeline (all matmuls f16, fp32 PSUM):
  The KV projection and attention stream together over 65 chunks of 128 kv
  tokens; K/V never spill to DRAM. Per chunk c: K then V projection (32 hc
  matmuls each, N=512 covering the 4 local kv heads), K rmsnorm+rope on
  scalar/vector (norm weights folded into host-precomputed rope tables),
  PE transpose to kT, then (lagged 8 chunks behind) S^T = kT_g.T @ qT4[g],
  P = exp(S^T) with no max subtraction, O^T += V_g.T @ P accumulated in
  PSUM over chunk pairs and drained to SBUF f32 by the vector engine.
  The softmax denominator l is accumulated elementwise on the vector engine
  (l_acc += P per chunk) and reduced over partitions once at the end with a
  single ones-vector matmul per group - no per-chunk PE work for l.
  Q projection (weights streamed in 8 half-group tiles during chunks 0-7)
  is emitted between chunks 7 and 8.
  Tail: aT = O * (1/l) via ones-outer broadcast matmul, pairwise AllGather,
  out = attnT.T @ WoT with Wo tiles streamed into the dead wku/wvu/xtu
  buffers plus a small tail pool.
"""
import numpy as np

import concourse.mybir as mybir
import concourse.tile as tile
from concourse import bacc
from concourse.bass_utils import run_bass_kernel_spmd
from concourse.masks import make_identity

B, Q_LEN, CTX, HID = 4, 128, 8192, 4096
H, KVH, D = 32, 8, 128
EPS = 1e-6
N_CORES = 8
TP = 2
HL, GL = H // TP, KVH // TP  # 16 local q heads, 4 local kv heads
AI = HL * D                  # 2048 local attention dims
G4 = GL * D                  # 512 = packed 4-head width

F16 = mybir.dt.float16
F32 = mybir.dt.float32
ALU = mybir.AluOpType
ACTF = mybir.ActivationFunctionType

HC = HID // 128              # 32 hidden chunks
LSC = 1.0 / 8192.0           # l rescale so 1/l stays in f16 normal range
LAG = 12                     # S runs LAG chunks behind the KV projection


def build(ctx_len=CTX):
    kv_len = ctx_len + Q_LEN
    assert ctx_len % 128 == 0
    n_ctx = ctx_len // 128           # 64 chunks from target_hidden
    n_chunks = kv_len // 128         # 65 total (last chunk = hidden_states)

    nc = bacc.Bacc("TRN2", target_bir_lowering=False, debug=False,
                   num_devices=N_CORES)

    # xs/tTs/wqs are host-side pre-arranged into the exact SBUF image
    # ([partition, (hc, col)]) so every DMA line is a multi-KB contiguous run
    xs = nc.dram_tensor("xs", [128, HC * Q_LEN], F16, kind="ExternalInput")
    tTs = nc.dram_tensor("tTs", [(ctx_len // 128) * 128, HC * 128], F16,
                         kind="ExternalInput")
    wqs = nc.dram_tensor("wqs", [16 * 128, HC * 128], F16,
                         kind="ExternalInput")
    wkT = nc.dram_tensor("wkT", [HID, G4], F16, kind="ExternalInput")
    wvT = nc.dram_tensor("wvT", [HID, G4], F16, kind="ExternalInput")
    woT = nc.dram_tensor("woT", [HID, AI], F16, kind="ExternalInput")
    cosq4 = nc.dram_tensor("cosq4", [Q_LEN, 512], F16, kind="ExternalInput")
    sinq4 = nc.dram_tensor("sinq4", [Q_LEN, 512], F16, kind="ExternalInput")
    cosk4 = nc.dram_tensor("cosk4", [kv_len, 512], F16, kind="ExternalInput")
    sink4 = nc.dram_tensor("sink4", [kv_len, 512], F16, kind="ExternalInput")
    y = nc.dram_tensor("y", [Q_LEN, AI], F32, kind="ExternalOutput")

    eps_ap = [None]

    def rope_block(pool, kfp, src_ap, cos_ap, sin_ap, tag):
        """src [128, 512] f32 (PSUM) -> rms-normed + roped f16 [128, 512].
        cos/sin are f16 [128, 512] with norm weights pre-folded.
        Squares and scaled copies run on the vector engine so the scalar
        engine's activation table (Sqrt/Exp/Copy) stays resident."""
        ksb = pool.tile([128, 512], F16, tag=f"{tag}ksb", name="ksb")
        nc.scalar.copy(ksb[:], src_ap)
        scr = pool.tile([128, 128], F32, tag=f"{tag}scr", name="scr")
        ms4 = pool.tile([128, 4], F32, tag=f"{tag}ms4", name="ms4")
        for g in range(4):
            sl = ksb[:, 128 * g:128 * (g + 1)]
            nc.vector.tensor_mul(scr[:], sl, sl)
            nc.vector.tensor_reduce(ms4[:, g:g + 1], scr[:],
                                    axis=mybir.AxisListType.X, op=ALU.add)
        rms4 = pool.tile([128, 4], F32, tag=f"{tag}rms4", name="rms4")
        nc.scalar.activation(rms4[:], ms4[:], func=ACTF.Sqrt,
                             scale=1.0 / D, bias=eps_ap[0])
        inv4 = pool.tile([128, 4], F32, tag=f"{tag}inv4", name="inv4")
        nc.vector.reciprocal(inv4[:], rms4[:])
        kn = pool.tile([128, 512], F16, tag=f"{tag}kn", name="kn")
        for g in range(4):
            nc.vector.tensor_scalar(kn[:, 128 * g:128 * (g + 1)],
                                    ksb[:, 128 * g:128 * (g + 1)],
                                    inv4[:, g:g + 1], None, op0=ALU.mult)
        knv = kn[:].rearrange("p (g d) -> p g d", d=D)
        sinv = sin_ap.rearrange("p (g d) -> p g d", d=D)
        t1 = pool.tile([128, 512], F16, tag=f"{tag}t1", name="t1")
        nc.vector.tensor_mul(t1[:], kn[:], cos_ap)
        t2 = pool.tile([128, 512], F16, tag=f"{tag}t2", name="t2")
        t2v = t2[:].rearrange("p (g d) -> p g d", d=D)
        nc.vector.scalar_tensor_tensor(
            t2v[:, :, 0:64], knv[:, :, 64:128], -1.0, sinv[:, :, 0:64],
            op0=ALU.mult, op1=ALU.mult)
        nc.vector.tensor_mul(t2v[:, :, 64:128], knv[:, :, 0:64],
                             sinv[:, :, 64:128])
        kf = kfp.tile([128, 512], F16, tag=f"{tag}kf", name="kf")
        nc.vector.tensor_add(kf[:], t1[:], t2[:])
        return kf

    with tile.TileContext(nc) as tc:
        with (
            tc.tile_pool(name="dram", bufs=1, space="DRAM") as dpool,
            tc.tile_pool(name="const", bufs=1) as cpool,
            tc.tile_pool(name="qt", bufs=1) as qtpool,
            tc.tile_pool(name="oacc", bufs=1) as opool,
        ):
            ag_in = dpool.tile([G4, 512], F16)
            ag_out = dpool.tile([TP * G4, 512], F16)

            cq_t = cpool.tile([Q_LEN, 512], F16)
            sq_t = cpool.tile([Q_LEN, 512], F16)
            ident = cpool.tile([128, 128], F16)
            ones = cpool.tile([128, 128], F16)
            eps_t = cpool.tile([128, 1], F32)

            qT4 = [qtpool.tile([D, 512], F16, name=f"qT4_{g}")
                   for g in range(GL)]
            o_acc = [opool.tile([D, 512], F32, name=f"oacc{g}")
                     for g in range(GL)]
            l_acc = [opool.tile([128, 512], F32, name=f"lacc{g}")
                     for g in range(GL)]

            with tc.tile_pool(name="xw", bufs=1) as xwpool:
                xtu = xwpool.tile([128, HC * Q_LEN], F16, name="xtu")
                wku = xwpool.tile([128, HC * G4], F16, name="wku")
                wvu = xwpool.tile([128, HC * G4], F16, name="wvu")
                xv = xtu[:].rearrange("p (c q) -> p c q", q=Q_LEN)
                wkv_ = wku[:].rearrange("p (c w) -> p c w", w=G4)
                wvv = wvu[:].rearrange("p (c w) -> p c w", w=G4)

                # state threaded between emission helpers
                kfs, kt4s, vsbs, pts = {}, {}, {}, {}
                ops_cur = {}

                with (
                    tc.tile_pool(name="ktb", bufs=LAG + 1) as ktb,
                    tc.tile_pool(name="vb", bufs=LAG + 2) as vbp,
                    tc.tile_pool(name="pt", bufs=9) as ptp,
                    tc.tile_pool(name="rp", bufs=2) as rp,
                    tc.tile_pool(name="kfp", bufs=3) as kfp,
                    tc.tile_pool(name="tt", bufs=2) as ttp,
                    tc.tile_pool(name="cs", bufs=2) as csp,
                    tc.tile_pool(name="kvp", bufs=1, space="PSUM") as kvpp,
                    tc.tile_pool(name="ktp", bufs=1, space="PSUM") as ktpp,
                ):
                    # ---------------- prologue ----------------
                    nc.sync.dma_start(xtu[:], xs[:, :])
                    nc.sync.dma_start(cq_t[:], cosq4[:, :])
                    nc.sync.dma_start(sq_t[:], sinq4[:, :])
                    make_identity(nc, ident[:])
                    nc.vector.memset(ones[:], 1.0)
                    nc.vector.memset(eps_t[:], EPS)
                    eps_ap[0] = eps_t[:]
                    for g in range(GL):
                        nc.vector.memset(o_acc[g][:], 0.0)
                        nc.vector.memset(l_acc[g][:], 0.0)
                    # wk/wv in interleaved pieces so chunk 0 starts early
                    for piece in range(4):
                        hc0, hc1 = 8 * piece, 8 * (piece + 1)
                        nc.sync.dma_start(
                            wkv_[:, hc0:hc1, :],
                            wkT[128 * hc0:128 * hc1, :].rearrange(
                                "(c p) w -> p c w", p=128))
                        nc.sync.dma_start(
                            wvv[:, hc0:hc1, :],
                            wvT[128 * hc0:128 * hc1, :].rearrange(
                                "(c p) w -> p c w", p=128))

                    def emit_kv(c):
                        if c < n_ctx:
                            ttu = ttp.tile([128, HC * 128], F16, tag="tt",
                                           name="ttu")
                            nc.sync.dma_start(
                                ttu[:], tTs[128 * c:128 * (c + 1), :])
                            src = ttu[:].rearrange("p (h w) -> p h w", w=128)
                        else:
                            src = xv
                        ck = csp.tile([128, 512], F16, tag="ck", name="ck")
                        nc.sync.dma_start(
                            ck[:], cosk4[128 * c:128 * (c + 1), :])
                        sk = csp.tile([128, 512], F16, tag="sk", name="sk")
                        nc.sync.dma_start(
                            sk[:], sink4[128 * c:128 * (c + 1), :])
                        kps = kvpp.tile([128, G4], F32, tag="kps", name="kps")
                        for hc in range(HC):
                            nc.tensor.matmul(kps[:], src[:, hc, :],
                                             wkv_[:, hc, :],
                                             start=(hc == 0),
                                             stop=(hc == HC - 1))
                        vps = kvpp.tile([128, G4], F32, tag="vps", name="vps")
                        for hc in range(HC):
                            nc.tensor.matmul(vps[:], src[:, hc, :],
                                             wvv[:, hc, :],
                                             start=(hc == 0),
                                             stop=(hc == HC - 1))
                        kfs[c] = rope_block(rp, kfp, kps[:], ck[:], sk[:],
                                            "k")
                        vsb = vbp.tile([128, G4], F16, tag="vsb", name="vsb")
                        nc.scalar.copy(vsb[:], vps[:])
                        vsbs[c] = vsb

                    def emit_T(c):
                        ktps = ktpp.tile([D, 512], F16, tag="ktps",
                                         name="ktps")
                        kf = kfs.pop(c)
                        for g in range(4):
                            nc.tensor.transpose(
                                ktps[:, 128 * g:128 * (g + 1)],
                                kf[:, 128 * g:128 * (g + 1)], ident[:])
                        kt4 = ktb.tile([D, 512], F16, tag="kt4", name="kt4")
                        nc.scalar.copy(kt4[:], ktps[:])
                        kt4s[c] = kt4

                    def emit_S(c, gs):
                        for g in gs:
                            sp = stp.tile([128, 512], F32, tag="stp",
                                          name="sp")
                            nc.tensor.matmul(
                                sp[:], kt4s[c][:, 128 * g:128 * (g + 1)],
                                qT4[g][:], start=True, stop=True)
                            pt = ptp.tile([128, 512], F16, tag="pt",
                                          name="pt")
                            nc.scalar.activation(pt[:], sp[:], func=ACTF.Exp)
                            nc.vector.tensor_add(l_acc[g][:], l_acc[g][:],
                                                 pt[:])
                            pts[(c, g)] = pt
                        if gs[-1] == GL - 1:
                            kt4s.pop(c)

                    def emit_O(c, gs):
                        if c < 0:
                            return
                        start = (c % 4 == 0)
                        stop = (c % 4 == 3) or (c == n_chunks - 1)
                        for g in gs:
                            if start:
                                ops_cur[g] = opp.tile([D, 512], F32,
                                                      tag="ops", name="ops")
                            nc.tensor.matmul(
                                ops_cur[g][:],
                                vsbs[c][:, 128 * g:128 * (g + 1)],
                                pts.pop((c, g))[:], start=start, stop=stop)
                            if stop:
                                nc.vector.tensor_add(o_acc[g][:],
                                                     o_acc[g][:],
                                                     ops_cur[g][:])
                        if gs[-1] == GL - 1:
                            vsbs.pop(c)

                    # ---------------- warmup chunks + Q ----------------
                    # Wq streams as 16 quarter-group tiles on the otherwise
                    # idle gpsimd DMA queue (ring waits there cannot block
                    # the tt/cos/sin stream on the sync queue). Q-projection
                    # groups are interleaved after warmup chunks 4..7 so the
                    # ring drains while the PE works.
                    wq_tiles = {}
                    with (
                        tc.tile_pool(name="wq", bufs=4) as wqp,
                        tc.tile_pool(name="qps", bufs=2, space="PSUM") as qpp,
                        tc.tile_pool(name="qtp", bufs=2, space="PSUM") as qtp,
                    ):
                        def load_wq(i):
                            t = wqp.tile([128, HC * 128], F16, tag="wq",
                                         name="wqh")
                            nc.gpsimd.dma_start(
                                t[:], wqs[128 * i:128 * (i + 1), :])
                            wq_tiles[i] = t

                        def emit_Q(g):
                            # wq tiles are hc-major: tile (4g+i) holds hc
                            # range [8i, 8i+8) x the full 512 group width,
                            # so every Q matmul runs at N=512
                            qps = qpp.tile([Q_LEN, 512], F32, tag="qps",
                                           name="qps")
                            for i in range(4):
                                wqv = wq_tiles[4 * g + i][:].rearrange(
                                    "p (c w) -> p c w", w=512)
                                for hl in range(8):
                                    hc = 8 * i + hl
                                    nc.tensor.matmul(
                                        qps[:], xv[:, hc, :],
                                        wqv[:, hl, :],
                                        start=(hc == 0),
                                        stop=(hc == HC - 1))
                            qf = rope_block(rp, kfp, qps[:], cq_t[:],
                                            sq_t[:], "q")
                            qtps = qtp.tile([D, 512], F16, tag="qtps",
                                            name="qtps")
                            for hh in range(4):
                                nc.tensor.transpose(
                                    qtps[:, 128 * hh:128 * (hh + 1)],
                                    qf[:, 128 * hh:128 * (hh + 1)],
                                    ident[:])
                            nc.scalar.copy(qT4[g][:], qtps[:])

                        wq_sched = {0: [0, 1, 2, 3]}
                        for c in range(1, 12):
                            wq_sched[c] = [c + 3] + ([15] if c == 11 else [])
                        for c in range(LAG):
                            emit_kv(c)
                            for i in wq_sched.get(c, []):
                                load_wq(i)
                            if c >= 1:
                                emit_T(c - 1)
                            if c >= LAG - 4:
                                emit_Q(c - (LAG - 4))

                    # ---------------- fused steady loop ----------------
                    with (
                        tc.tile_pool(name="stp", bufs=3,
                                     space="PSUM") as stp,
                        tc.tile_pool(name="opp", bufs=2,
                                     space="PSUM") as opp,
                    ):
                        for c in range(LAG, n_chunks):
                            emit_kv(c)
                            emit_T(c - 1)
                            emit_S(c - LAG, (0, 1))
                            emit_O(c - LAG - 1, (0, 1))
                            emit_S(c - LAG, (2, 3))
                            emit_O(c - LAG - 1, (2, 3))
                        # catchup: attention for the last LAG chunks
                        emit_T(n_chunks - 1)
                        for c in range(n_chunks - LAG, n_chunks):
                            emit_S(c, (0, 1))
                            emit_O(c - 1, (0, 1))
                            emit_S(c, (2, 3))
                            emit_O(c - 1, (2, 3))
                        emit_O(n_chunks - 1, (0, 1))
                        emit_O(n_chunks - 1, (2, 3))

                # ---------------- tail: scale, allgather, Wo ----------
                # Wo tiles stream into the dead wku/wvu/xtu buffers
                # (8 + 8 + 2 head-blocks) plus a 14-tile tail pool.
                NH = TP * HL
                NJ = AI // 512
                slot_views = []
                for t in (wku, wvu, xtu):
                    v = t[:].rearrange("p (s w) -> p s w", w=AI)
                    for s in range(v.shape[1]):
                        slot_views.append(v[:, s, :])
                with (
                    tc.tile_pool(name="wot", bufs=14) as wotp,
                    tc.tile_pool(name="tsb", bufs=2) as tsb,
                    tc.tile_pool(name="agp", bufs=1) as agp,
                    tc.tile_pool(name="wps", bufs=1, space="PSUM") as wopp,
                    tc.tile_pool(name="lsp", bufs=1, space="PSUM") as lsp,
                    tc.tile_pool(name="bcps", bufs=2, space="PSUM") as bcpp,
                ):
                    wo_aps = {}

                    def load_wo(h, eng):
                        if h < 18:
                            dst = slot_views[h]
                        else:
                            t = wotp.tile([128, AI], F16, tag="wo",
                                          name="wot")
                            dst = t[:]
                        eng.dma_start(dst, woT[128 * h:128 * (h + 1), :])
                        wo_aps[h] = dst

                    # slot loads reuse the dead wku/wvu/xtu buffers; they
                    # stream on the gpsimd queue during the catchup chunks
                    for h in range(18):
                        load_wo(h, nc.gpsimd)

                    # broadcast l across partitions first (ones-outer
                    # matmul), then reciprocal on the full [128, 512] tile -
                    # a [1, 512] reciprocal would run on a single DVE lane
                    # stage-interleaved so the four groups' chains pipeline
                    # across scalar/PE/vector instead of running serially
                    aT4 = [tsb.tile([D, 512], F16, tag=f"aT4_{g}",
                                    name=f"aT4_{g}") for g in range(GL)]
                    l16s, lsbs = [], []
                    for g in range(GL):
                        l16 = tsb.tile([128, 512], F16, tag=f"l16_{g}",
                                       name="l16")
                        nc.scalar.copy(l16[:], l_acc[g][:])
                        l16s.append(l16)
                    for g in range(GL):
                        lsum = lsp.tile([1, 512], F32, tag="lsum",
                                        name="lsum")
                        nc.tensor.matmul(lsum[:], ones[:, 0:1], l16s[g][:],
                                         start=True, stop=True)
                        lsb = tsb.tile([1, 512], F16, tag=f"lsb_{g}",
                                       name="lsb")
                        nc.scalar.copy(lsb[:], lsum[:])
                        lsbs.append(lsb)
                    for g in range(GL):
                        bcl = bcpp.tile([128, 512], F32, tag="bcl",
                                        name="bcl")
                        for hh in range(4):
                            cs = slice(128 * hh, 128 * (hh + 1))
                            nc.tensor.matmul(bcl[:, cs], ones[0:1, :],
                                             lsbs[g][:, cs], start=True,
                                             stop=True)
                        invb = tsb.tile([128, 512], F32, tag="invb",
                                        name="invb")
                        nc.vector.reciprocal(invb[:], bcl[:])
                        nc.vector.scalar_tensor_tensor(
                            aT4[g][:], o_acc[g][:], 1.0, invb[:],
                            op0=ALU.mult, op1=ALU.mult)
                        nc.sync.dma_start(
                            ag_in[128 * g:128 * (g + 1), :], aT4[g][:])
                    nc.gpsimd.collective_compute(
                        "AllGather",
                        ALU.bypass,
                        replica_groups=[[0, 1], [2, 3], [4, 5], [6, 7]],
                        ins=[ag_in.opt()],
                        outs=[ag_out.opt()],
                    )
                    # remaining Wo tiles stream on the scalar queue (idle
                    # in the tail) so the ag DMAs on sync aren't blocked
                    for h in range(18, NH):
                        load_wo(h, nc.scalar)
                    ag_sb = []
                    for t in range(TP):
                        a_sb = agp.tile([128, GL * 512], F16,
                                        name=f"ag_sb{t}")
                        nc.sync.dma_start(
                            a_sb[:].rearrange("p (g w) -> p g w", w=512),
                            ag_out[G4 * t:G4 * (t + 1), :].rearrange(
                                "(g p) w -> p g w", p=128))
                        ag_sb.append(a_sb)
                    wps = [wopp.tile([Q_LEN, 512], F32, tag=f"wps{j}",
                                     name="wps") for j in range(NJ)]
                    for h in range(NH):
                        t, g, hh = h // 16, (h % 16) // 4, h % 4
                        lhs = ag_sb[t][:].rearrange("p (g w) -> p g w",
                                                    w=512)
                        for j in range(NJ):
                            nc.tensor.matmul(
                                wps[j][:],
                                lhs[:, g, 128 * hh:128 * (hh + 1)],
                                wo_aps[h][:, 512 * j:512 * (j + 1)],
                                start=(h == 0), stop=(h == NH - 1))
                    for j in range(NJ):
                        out_j = tsb.tile([Q_LEN, 512], F32, tag="outj",
                                         name="outj")
                        nc.vector.tensor_copy(out_j[:], wps[j][:])
                        nc.sync.dma_start(y[:, 512 * j:512 * (j + 1)],
                                          out_j[:])

    nc.compile()
    return nc


def host_prep(hidden_states, target_hidden, cos, sin, Wq, Wk, Wv, Wo,
              q_norm_w, k_norm_w, ctx_len=CTX):
    """Build the 8 per-core input maps from full inputs (numpy, host side)."""
    kv_len = ctx_len + Q_LEN
    f16 = np.float16
    f32 = np.float32

    qw = np.asarray(q_norm_w, f32)
    kw = np.asarray(k_norm_w, f32)
    qw_rot = np.concatenate([qw[64:], qw[:64]])
    kw_rot = np.concatenate([kw[64:], kw[:64]])
    scale = np.float32(D ** -0.5)

    per_b = {}
    for b in range(B):
        cq = np.asarray(cos[b, ctx_len:kv_len], f32)
        sq = np.asarray(sin[b, ctx_len:kv_len], f32)
        ckv = np.asarray(cos[b, :kv_len], f32)
        skv = np.asarray(sin[b, :kv_len], f32)
        xs = (np.asarray(hidden_states[b], f32).T.astype(f16)
              .reshape(HC, 128, Q_LEN).transpose(1, 0, 2).reshape(128, -1))
        tts = (np.asarray(target_hidden[b], f32).astype(f16)
               .reshape(ctx_len // 128, 128, HC, 128)
               .transpose(0, 3, 2, 1).reshape(-1, HC * 128))
        per_b[b] = {
            "xs": np.ascontiguousarray(xs),
            "tTs": np.ascontiguousarray(tts),
            "cosq4": np.ascontiguousarray(
                np.tile((cq * qw * scale).astype(f16), (1, 4))),
            "sinq4": np.ascontiguousarray(
                np.tile((sq * qw_rot * scale).astype(f16), (1, 4))),
            "cosk4": np.ascontiguousarray(
                np.tile((ckv * kw).astype(f16), (1, 4))),
            "sink4": np.ascontiguousarray(
                np.tile((skv * kw_rot).astype(f16), (1, 4))),
        }
    per_t = {}
    for t in range(TP):
        wq_t = np.asarray(Wq[AI * t:AI * (t + 1), :], f32).astype(f16)
        wqs = (wq_t.reshape(4, 512, 4, 8, 128).transpose(0, 2, 4, 3, 1)
               .reshape(-1, 8 * 512))
        per_t[t] = {
            "wqs": np.ascontiguousarray(wqs),
            "wkT": np.ascontiguousarray(
                Wk[G4 * t:G4 * (t + 1), :].T).astype(f16),
            "wvT": np.ascontiguousarray(
                Wv[G4 * t:G4 * (t + 1), :].T).astype(f16),
            "woT": np.ascontiguousarray(
                Wo[AI * t:AI * (t + 1), :].T).astype(f16),
        }
    in_maps = []
    for c in range(N_CORES):
        b, t = c // TP, c % TP
        m = {}
        m.update(per_b[b])
        m.update(per_t[t])
        in_maps.append(m)
    return in_maps


_CACHE = {}


def _get_module(ctx_len=CTX):
    if ctx_len not in _CACHE:
        _CACHE[ctx_len] = build(ctx_len)
    return _CACHE[ctx_len]


def kernel(hidden_states, target_hidden, cos, sin, Wq, Wk, Wv, Wo,
           q_norm_w, k_norm_w):
    args = [np.asarray(a) for a in (hidden_states, target_hidden, cos, sin,
                                    Wq, Wk, Wv, Wo, q_norm_w, k_norm_w)]
    nc = _get_module(CTX)
    in_maps = host_prep(*args, ctx_len=CTX)
    res = run_bass_kernel_spmd(nc, in_maps, core_ids=list(range(N_CORES)))
    out = np.stack(
        [np.concatenate([res.results[TP * b + t]["y"] for t in range(TP)],
                        axis=1) for b in range(B)], axis=0)
    return out.astype(np.float32)



# revision 9
# speedup vs baseline: 44.7618x; 44.7618x over previous
"""DFlashAttention on 8 Trainium2 NeuronCores.

Sharding: data-parallel over batch (4) x tensor-parallel over heads (2).
Core c = 2*b + t handles batch b, q heads [16t, 16t+16), kv heads [4t, 4t+4).
GQA groups (4 q heads per kv head) align with the head split, so attention is
core-local. Output projection: AllGather of attention outputs over the TP
pair, then each core computes its [128, 2048] output column block.

Fused single-pass pipeline (all matmuls f16, fp32 PSUM):
  The KV projection and attention stream together over 65 chunks of 128 kv
  tokens; K/V never spill to DRAM. Per chunk c: K then V projection (32 hc
  matmuls each, N=512 covering the 4 local kv heads), K rmsnorm+rope on
  scalar/vector (norm weights folded into host-precomputed rope tables),
  PE transpose to kT, then (lagged 8 chunks behind) S^T = kT_g.T @ qT4[g],
  P = exp(S^T) with no max subtraction, O^T += V_g.T @ P accumulated in
  PSUM over chunk pairs and drained to SBUF f32 by the vector engine.
  The softmax denominator l is accumulated elementwise on the vector engine
  (l_acc += P per chunk) and reduced over partitions once at the end with a
  single ones-vector matmul per group - no per-chunk PE work for l.
  Q projection (weights streamed in 8 half-group tiles during chunks 0-7)
  is emitted between chunks 7 and 8.
  Tail: aT = O * (1/l) via ones-outer broadcast matmul, pairwise AllGather,
  out = attnT.T @ WoT with Wo tiles streamed into the dead wku/wvu/xtu
  buffers plus a small tail pool.
"""
import numpy as np

import concourse.mybir as mybir
import concourse.tile as tile
from concourse import bacc
from concourse.bass_utils import run_bass_kernel_spmd
from concourse.masks import make_identity

B, Q_LEN, CTX, HID = 4, 128, 8192, 4096
H, KVH, D = 32, 8, 128
EPS = 1e-6
N_CORES = 8
TP = 2
HL, GL = H // TP, KVH // TP  # 16 local q heads, 4 local kv heads
AI = HL * D                  # 2048 local attention dims
G4 = GL * D                  # 512 = packed 4-head width

F16 = mybir.dt.bfloat16
F32 = mybir.dt.float32
ALU = mybir.AluOpType
ACTF = mybir.ActivationFunctionType

HC = HID // 128              # 32 hidden chunks
LSC = 1.0 / 8192.0           # l rescale so 1/l stays in f16 normal range
LAG = 12                     # S runs LAG chunks behind the KV projection


def build(ctx_len=CTX, timing=False):
    kv_len = ctx_len + Q_LEN
    assert ctx_len % 128 == 0
    n_ctx = ctx_len // 128           # 64 chunks from target_hidden
    n_chunks = kv_len // 128         # 65 total (last chunk = hidden_states)

    nc = bacc.Bacc("TRN2", target_bir_lowering=False, debug=False,
                   num_devices=N_CORES)

    # timing=True turns the big inputs into Internal DRAM scratch (garbage
    # data, identical instruction stream) so per-call input shipping through
    # the axon tunnel vanishes and wall-minus-null resolves HW exec time.
    kb = "Internal" if timing else "ExternalInput"

    # xs/tTs/wqs are host-side pre-arranged into the exact SBUF image
    # ([partition, (hc, col)]) so every DMA line is a multi-KB contiguous run
    xs = nc.dram_tensor("xs", [128, HC * Q_LEN], F16, kind=kb)
    tTs = nc.dram_tensor("tTs", [(ctx_len // 128) * 128, HC * 128], F16,
                         kind=kb)
    wqs = nc.dram_tensor("wqs", [16 * 128, HC * 128], F16,
                         kind=kb)
    wkT = nc.dram_tensor("wkT", [HID, G4], F16, kind=kb)
    wvT = nc.dram_tensor("wvT", [HID, G4], F16, kind=kb)
    woT = nc.dram_tensor("woT", [HID, AI], F16, kind=kb)
    cosq4 = nc.dram_tensor("cosq4", [Q_LEN, 512], F16, kind=kb)
    sinq4 = nc.dram_tensor("sinq4", [Q_LEN, 512], F16, kind=kb)
    cosk4 = nc.dram_tensor("cosk4", [kv_len, 512], F16, kind=kb)
    sink4 = nc.dram_tensor("sink4", [kv_len, 512], F16, kind=kb)
    y = nc.dram_tensor("y", [Q_LEN, AI], F32, kind="ExternalOutput")
    # 1x1 passthrough so chained timing execs have a data dependence
    # (the bass_exec custom_call is CSE-able; see time_hw.py)
    chain_in = nc.dram_tensor("chain_in", [1, 1], F32, kind="ExternalInput")
    chain_out = nc.dram_tensor("chain_out", [1, 1], F32, kind="ExternalOutput")

    eps_ap = [None]

    def rope_block(pool, kfp, src_ap, cos_ap, sin_ap, tag):
        """src [128, 512] f32 (PSUM) -> rms-normed + roped f16 [128, 512].
        cos/sin are f16 [128, 512] with norm weights pre-folded.
        Squares and scaled copies run on the vector engine so the scalar
        engine's activation table (Sqrt/Exp/Copy) stays resident."""
        ksb = pool.tile([128, 512], F16, tag=f"{tag}ksb", name="ksb")
        nc.scalar.copy(ksb[:], src_ap)
        scr = pool.tile([128, 128], F32, tag=f"{tag}scr", name="scr")
        ms4 = pool.tile([128, 4], F32, tag=f"{tag}ms4", name="ms4")
        for g in range(4):
            sl = ksb[:, 128 * g:128 * (g + 1)]
            nc.vector.tensor_mul(scr[:], sl, sl)
            nc.vector.tensor_reduce(ms4[:, g:g + 1], scr[:],
                                    axis=mybir.AxisListType.X, op=ALU.add)
        rms4 = pool.tile([128, 4], F32, tag=f"{tag}rms4", name="rms4")
        nc.scalar.activation(rms4[:], ms4[:], func=ACTF.Sqrt,
                             scale=1.0 / D, bias=eps_ap[0])
        inv4 = pool.tile([128, 4], F32, tag=f"{tag}inv4", name="inv4")
        nc.vector.reciprocal(inv4[:], rms4[:])
        kn = pool.tile([128, 512], F16, tag=f"{tag}kn", name="kn")
        for g in range(4):
            nc.vector.tensor_scalar(kn[:, 128 * g:128 * (g + 1)],
                                    ksb[:, 128 * g:128 * (g + 1)],
                                    inv4[:, g:g + 1], None, op0=ALU.mult)
        knv = kn[:].rearrange("p (g d) -> p g d", d=D)
        sinv = sin_ap.rearrange("p (g d) -> p g d", d=D)
        t1 = pool.tile([128, 512], F16, tag=f"{tag}t1", name="t1")
        nc.vector.tensor_mul(t1[:], kn[:], cos_ap)
        t2 = pool.tile([128, 512], F16, tag=f"{tag}t2", name="t2")
        t2v = t2[:].rearrange("p (g d) -> p g d", d=D)
        nc.vector.scalar_tensor_tensor(
            t2v[:, :, 0:64], knv[:, :, 64:128], -1.0, sinv[:, :, 0:64],
            op0=ALU.mult, op1=ALU.mult)
        nc.vector.tensor_mul(t2v[:, :, 64:128], knv[:, :, 0:64],
                             sinv[:, :, 64:128])
        kf = kfp.tile([128, 512], F16, tag=f"{tag}kf", name="kf")
        nc.vector.tensor_add(kf[:], t1[:], t2[:])
        return kf

    with tile.TileContext(nc) as tc:
        with (
            tc.tile_pool(name="dram", bufs=1, space="DRAM") as dpool,
            tc.tile_pool(name="const", bufs=1) as cpool,
            tc.tile_pool(name="qt", bufs=1) as qtpool,
            tc.tile_pool(name="oacc", bufs=1) as opool,
        ):
            ag_in = dpool.tile([G4, 512], F16)
            ag_out = dpool.tile([TP * G4, 512], F16)

            cq_t = cpool.tile([Q_LEN, 512], F16)
            sq_t = cpool.tile([Q_LEN, 512], F16)
            ident = cpool.tile([128, 128], F16)
            ones = cpool.tile([128, 128], F16)
            eps_t = cpool.tile([128, 1], F32)
            chn_t = cpool.tile([1, 1], F32)
            nc.scalar.dma_start(chn_t[:], chain_in[:, :])
            nc.scalar.dma_start(chain_out[:, :], chn_t[:])

            qT4 = [qtpool.tile([D, 512], F16, name=f"qT4_{g}")
                   for g in range(GL)]
            o_acc = [opool.tile([D, 512], F32, name=f"oacc{g}")
                     for g in range(GL)]
            l_acc = [opool.tile([128, 512], F32, name=f"lacc{g}")
                     for g in range(GL)]

            with tc.tile_pool(name="xw", bufs=1) as xwpool:
                xtu = xwpool.tile([128, HC * Q_LEN], F16, name="xtu")
                wku = xwpool.tile([128, HC * G4], F16, name="wku")
                wvu = xwpool.tile([128, HC * G4], F16, name="wvu")
                xv = xtu[:].rearrange("p (c q) -> p c q", q=Q_LEN)
                wkv_ = wku[:].rearrange("p (c w) -> p c w", w=G4)
                wvv = wvu[:].rearrange("p (c w) -> p c w", w=G4)

                # state threaded between emission helpers
                kfs, kt4s, vsbs, pts = {}, {}, {}, {}
                ops_cur = {}

                with (
                    tc.tile_pool(name="ktb", bufs=LAG + 1) as ktb,
                    tc.tile_pool(name="vb", bufs=LAG + 2) as vbp,
                    tc.tile_pool(name="pt", bufs=9) as ptp,
                    tc.tile_pool(name="rp", bufs=2) as rp,
                    tc.tile_pool(name="kfp", bufs=3) as kfp,
                    tc.tile_pool(name="tt", bufs=2) as ttp,
                    tc.tile_pool(name="cs", bufs=2) as csp,
                    tc.tile_pool(name="kvp", bufs=1, space="PSUM") as kvpp,
                    tc.tile_pool(name="ktp", bufs=1, space="PSUM") as ktpp,
                ):
                    # ---------------- prologue ----------------
                    nc.sync.dma_start(xtu[:], xs[:, :])
                    nc.sync.dma_start(cq_t[:], cosq4[:, :])
                    nc.sync.dma_start(sq_t[:], sinq4[:, :])
                    make_identity(nc, ident[:])
                    nc.vector.memset(ones[:], 1.0)
                    nc.vector.memset(eps_t[:], EPS)
                    eps_ap[0] = eps_t[:]
                    for g in range(GL):
                        nc.vector.memset(o_acc[g][:], 0.0)
                        nc.vector.memset(l_acc[g][:], 0.0)
                    # wk/wv in interleaved pieces so chunk 0 starts early
                    for piece in range(4):
                        hc0, hc1 = 8 * piece, 8 * (piece + 1)
                        nc.sync.dma_start(
                            wkv_[:, hc0:hc1, :],
                            wkT[128 * hc0:128 * hc1, :].rearrange(
                                "(c p) w -> p c w", p=128))
                        nc.sync.dma_start(
                            wvv[:, hc0:hc1, :],
                            wvT[128 * hc0:128 * hc1, :].rearrange(
                                "(c p) w -> p c w", p=128))

                    def emit_kv(c):
                        if c < n_ctx:
                            ttu = ttp.tile([128, HC * 128], F16, tag="tt",
                                           name="ttu")
                            nc.sync.dma_start(
                                ttu[:], tTs[128 * c:128 * (c + 1), :])
                            src = ttu[:].rearrange("p (h w) -> p h w", w=128)
                        else:
                            src = xv
                        ck = csp.tile([128, 512], F16, tag="ck", name="ck")
                        nc.sync.dma_start(
                            ck[:], cosk4[128 * c:128 * (c + 1), :])
                        sk = csp.tile([128, 512], F16, tag="sk", name="sk")
                        nc.sync.dma_start(
                            sk[:], sink4[128 * c:128 * (c + 1), :])
                        kps = kvpp.tile([128, G4], F32, tag="kps", name="kps")
                        for hc in range(HC):
                            nc.tensor.matmul(kps[:], src[:, hc, :],
                                             wkv_[:, hc, :],
                                             start=(hc == 0),
                                             stop=(hc == HC - 1))
                        vps = kvpp.tile([128, G4], F32, tag="vps", name="vps")
                        for hc in range(HC):
                            nc.tensor.matmul(vps[:], src[:, hc, :],
                                             wvv[:, hc, :],
                                             start=(hc == 0),
                                             stop=(hc == HC - 1))
                        kfs[c] = rope_block(rp, kfp, kps[:], ck[:], sk[:],
                                            "k")
                        vsb = vbp.tile([128, G4], F16, tag="vsb", name="vsb")
                        nc.scalar.copy(vsb[:], vps[:])
                        vsbs[c] = vsb

                    def emit_T(c):
                        ktps = ktpp.tile([D, 512], F16, tag="ktps",
                                         name="ktps")
                        kf = kfs.pop(c)
                        for g in range(4):
                            nc.tensor.transpose(
                                ktps[:, 128 * g:128 * (g + 1)],
                                kf[:, 128 * g:128 * (g + 1)], ident[:])
                        kt4 = ktb.tile([D, 512], F16, tag="kt4", name="kt4")
                        nc.scalar.copy(kt4[:], ktps[:])
                        kt4s[c] = kt4

                    def emit_S(c, gs):
                        for g in gs:
                            sp = stp.tile([128, 512], F32, tag="stp",
                                          name="sp")
                            nc.tensor.matmul(
                                sp[:], kt4s[c][:, 128 * g:128 * (g + 1)],
                                qT4[g][:], start=True, stop=True)
                            pt = ptp.tile([128, 512], F16, tag="pt",
                                          name="pt")
                            nc.scalar.activation(pt[:], sp[:], func=ACTF.Exp)
                            nc.vector.tensor_add(l_acc[g][:], l_acc[g][:],
                                                 pt[:])
                            pts[(c, g)] = pt
                        if gs[-1] == GL - 1:
                            kt4s.pop(c)

                    def emit_O(c, gs):
                        if c < 0:
                            return
                        start = (c % 4 == 0)
                        stop = (c % 4 == 3) or (c == n_chunks - 1)
                        for g in gs:
                            if start:
                                ops_cur[g] = opp.tile([D, 512], F32,
                                                      tag="ops", name="ops")
                            nc.tensor.matmul(
                                ops_cur[g][:],
                                vsbs[c][:, 128 * g:128 * (g + 1)],
                                pts.pop((c, g))[:], start=start, stop=stop)
                            if stop:
                                nc.vector.tensor_add(o_acc[g][:],
                                                     o_acc[g][:],
                                                     ops_cur[g][:])
                        if gs[-1] == GL - 1:
                            vsbs.pop(c)

                    # ---------------- warmup chunks + Q ----------------
                    # Wq streams as 16 quarter-group tiles on the otherwise
                    # idle gpsimd DMA queue (ring waits there cannot block
                    # the tt/cos/sin stream on the sync queue). Q-projection
                    # groups are interleaved after warmup chunks 4..7 so the
                    # ring drains while the PE works.
                    wq_tiles = {}
                    with (
                        tc.tile_pool(name="wq", bufs=4) as wqp,
                        tc.tile_pool(name="qps", bufs=2, space="PSUM") as qpp,
                        tc.tile_pool(name="qtp", bufs=2, space="PSUM") as qtp,
                    ):
                        def load_wq(i):
                            t = wqp.tile([128, HC * 128], F16, tag="wq",
                                         name="wqh")
                            nc.gpsimd.dma_start(
                                t[:], wqs[128 * i:128 * (i + 1), :])
                            wq_tiles[i] = t

                        def emit_Q(g):
                            # wq tiles are hc-major: tile (4g+i) holds hc
                            # range [8i, 8i+8) x the full 512 group width,
                            # so every Q matmul runs at N=512
                            qps = qpp.tile([Q_LEN, 512], F32, tag="qps",
                                           name="qps")
                            for i in range(4):
                                wqv = wq_tiles[4 * g + i][:].rearrange(
                                    "p (c w) -> p c w", w=512)
                                for hl in range(8):
                                    hc = 8 * i + hl
                                    nc.tensor.matmul(
                                        qps[:], xv[:, hc, :],
                                        wqv[:, hl, :],
                                        start=(hc == 0),
                                        stop=(hc == HC - 1))
                            qf = rope_block(rp, kfp, qps[:], cq_t[:],
                                            sq_t[:], "q")
                            qtps = qtp.tile([D, 512], F16, tag="qtps",
                                            name="qtps")
                            for hh in range(4):
                                nc.tensor.transpose(
                                    qtps[:, 128 * hh:128 * (hh + 1)],
                                    qf[:, 128 * hh:128 * (hh + 1)],
                                    ident[:])
                            nc.scalar.copy(qT4[g][:], qtps[:])

                        wq_sched = {0: [0, 1, 2, 3]}
                        for c in range(1, 12):
                            wq_sched[c] = [c + 3] + ([15] if c == 11 else [])
                        for c in range(LAG):
                            emit_kv(c)
                            for i in wq_sched.get(c, []):
                                load_wq(i)
                            if c >= 1:
                                emit_T(c - 1)
                            if c >= LAG - 4:
                                emit_Q(c - (LAG - 4))

                    # ---------------- fused steady loop ----------------
                    with (
                        tc.tile_pool(name="stp", bufs=3,
                                     space="PSUM") as stp,
                        tc.tile_pool(name="opp", bufs=2,
                                     space="PSUM") as opp,
                    ):
                        for c in range(LAG, n_chunks):
                            emit_kv(c)
                            emit_T(c - 1)
                            emit_S(c - LAG, (0, 1))
                            emit_O(c - LAG - 1, (0, 1))
                            emit_S(c - LAG, (2, 3))
                            emit_O(c - LAG - 1, (2, 3))
                        # catchup: attention for the last LAG chunks
                        emit_T(n_chunks - 1)
                        for c in range(n_chunks - LAG, n_chunks):
                            emit_S(c, (0, 1))
                            emit_O(c - 1, (0, 1))
                            emit_S(c, (2, 3))
                            emit_O(c - 1, (2, 3))
                        emit_O(n_chunks - 1, (0, 1))
                        emit_O(n_chunks - 1, (2, 3))

                # ---------------- tail: scale, allgather, Wo ----------
                # Wo tiles stream into the dead wku/wvu/xtu buffers
                # (8 + 8 + 2 head-blocks) plus a 14-tile tail pool.
                NH = TP * HL
                NJ = AI // 512
                slot_views = []
                for t in (wku, wvu, xtu):
                    v = t[:].rearrange("p (s w) -> p s w", w=AI)
                    for s in range(v.shape[1]):
                        slot_views.append(v[:, s, :])
                with (
                    tc.tile_pool(name="wot", bufs=14) as wotp,
                    tc.tile_pool(name="tsb", bufs=2) as tsb,
                    tc.tile_pool(name="agp", bufs=1) as agp,
                    tc.tile_pool(name="wps", bufs=1, space="PSUM") as wopp,
                    tc.tile_pool(name="lsp", bufs=1, space="PSUM") as lsp,
                    tc.tile_pool(name="bcps", bufs=2, space="PSUM") as bcpp,
                ):
                    wo_aps = {}

                    def load_wo(h, eng):
                        if h < 18:
                            dst = slot_views[h]
                        else:
                            t = wotp.tile([128, AI], F16, tag="wo",
                                          name="wot")
                            dst = t[:]
                        eng.dma_start(dst, woT[128 * h:128 * (h + 1), :])
                        wo_aps[h] = dst

                    # slot loads reuse the dead wku/wvu/xtu buffers; they
                    # stream on the gpsimd queue during the catchup chunks
                    for h in range(18):
                        load_wo(h, nc.gpsimd)

                    # broadcast l across partitions first (ones-outer
                    # matmul), then reciprocal on the full [128, 512] tile -
                    # a [1, 512] reciprocal would run on a single DVE lane
                    # stage-interleaved so the four groups' chains pipeline
                    # across scalar/PE/vector instead of running serially
                    aT4 = [tsb.tile([D, 512], F16, tag=f"aT4_{g}",
                                    name=f"aT4_{g}") for g in range(GL)]
                    l16s, lsbs = [], []
                    for g in range(GL):
                        l16 = tsb.tile([128, 512], F16, tag=f"l16_{g}",
                                       name="l16")
                        nc.scalar.copy(l16[:], l_acc[g][:])
                        l16s.append(l16)
                    for g in range(GL):
                        lsum = lsp.tile([1, 512], F32, tag="lsum",
                                        name="lsum")
                        nc.tensor.matmul(lsum[:], ones[:, 0:1], l16s[g][:],
                                         start=True, stop=True)
                        lsb = tsb.tile([1, 512], F16, tag=f"lsb_{g}",
                                       name="lsb")
                        nc.scalar.copy(lsb[:], lsum[:])
                        lsbs.append(lsb)
                    for g in range(GL):
                        bcl = bcpp.tile([128, 512], F32, tag="bcl",
                                        name="bcl")
                        for hh in range(4):
                            cs = slice(128 * hh, 128 * (hh + 1))
                            nc.tensor.matmul(bcl[:, cs], ones[0:1, :],
                                             lsbs[g][:, cs], start=True,
                                             stop=True)
                        invb = tsb.tile([128, 512], F32, tag="invb",
                                        name="invb")
                        nc.vector.reciprocal(invb[:], bcl[:])
                        nc.vector.scalar_tensor_tensor(
                            aT4[g][:], o_acc[g][:], 1.0, invb[:],
                            op0=ALU.mult, op1=ALU.mult)
                        nc.sync.dma_start(
                            ag_in[128 * g:128 * (g + 1), :], aT4[g][:])
                    nc.gpsimd.collective_compute(
                        "AllGather",
                        ALU.bypass,
                        replica_groups=[[0, 1], [2, 3], [4, 5], [6, 7]],
                        ins=[ag_in.opt()],
                        outs=[ag_out.opt()],
                    )
                    # remaining Wo tiles stream on the scalar queue (idle
                    # in the tail) so the ag DMAs on sync aren't blocked
                    for h in range(18, NH):
                        load_wo(h, nc.scalar)
                    ag_sb = []
                    for t in range(TP):
                        a_sb = agp.tile([128, GL * 512], F16,
                                        name=f"ag_sb{t}")
                        nc.sync.dma_start(
                            a_sb[:].rearrange("p (g w) -> p g w", w=512),
                            ag_out[G4 * t:G4 * (t + 1), :].rearrange(
                                "(g p) w -> p g w", p=128))
                        ag_sb.append(a_sb)
                    wps = [wopp.tile([Q_LEN, 512], F32, tag=f"wps{j}",
                                     name="wps") for j in range(NJ)]
                    for h in range(NH):
                        t, g, hh = h // 16, (h % 16) // 4, h % 4
                        lhs = ag_sb[t][:].rearrange("p (g w) -> p g w",
                                                    w=512)
                        for j in range(NJ):
                            nc.tensor.matmul(
                                wps[j][:],
                                lhs[:, g, 128 * hh:128 * (hh + 1)],
                                wo_aps[h][:, 512 * j:512 * (j + 1)],
                                start=(h == 0), stop=(h == NH - 1))
                    for j in range(NJ):
                        out_j = tsb.tile([Q_LEN, 512], F32, tag="outj",
                                         name="outj")
                        nc.vector.tensor_copy(out_j[:], wps[j][:])
                        nc.sync.dma_start(y[:, 512 * j:512 * (j + 1)],
                                          out_j[:])

    nc.compile()
    return nc


def host_prep(hidden_states, target_hidden, cos, sin, Wq, Wk, Wv, Wo,
              q_norm_w, k_norm_w, ctx_len=CTX):
    """Build the 8 per-core input maps from full inputs (numpy, host side)."""
    kv_len = ctx_len + Q_LEN
    f16 = mybir.dt.np(mybir.dt.bfloat16)
    f32 = np.float32

    qw = np.asarray(q_norm_w, f32)
    kw = np.asarray(k_norm_w, f32)
    qw_rot = np.concatenate([qw[64:], qw[:64]])
    kw_rot = np.concatenate([kw[64:], kw[:64]])
    scale = np.float32(D ** -0.5)

    per_b = {}
    for b in range(B):
        cq = np.asarray(cos[b, ctx_len:kv_len], f32)
        sq = np.asarray(sin[b, ctx_len:kv_len], f32)
        ckv = np.asarray(cos[b, :kv_len], f32)
        skv = np.asarray(sin[b, :kv_len], f32)
        xs = (np.asarray(hidden_states[b], f32).T.astype(f16)
              .reshape(HC, 128, Q_LEN).transpose(1, 0, 2).reshape(128, -1))
        tts = (np.asarray(target_hidden[b], f32).astype(f16)
               .reshape(ctx_len // 128, 128, HC, 128)
               .transpose(0, 3, 2, 1).reshape(-1, HC * 128))
        per_b[b] = {
            "chain_in": np.zeros((1, 1), np.float32),
            "xs": np.ascontiguousarray(xs),
            "tTs": np.ascontiguousarray(tts),
            "cosq4": np.ascontiguousarray(
                np.tile((cq * qw * scale).astype(f16), (1, 4))),
            "sinq4": np.ascontiguousarray(
                np.tile((sq * qw_rot * scale).astype(f16), (1, 4))),
            "cosk4": np.ascontiguousarray(
                np.tile((ckv * kw).astype(f16), (1, 4))),
            "sink4": np.ascontiguousarray(
                np.tile((skv * kw_rot).astype(f16), (1, 4))),
        }
    per_t = {}
    for t in range(TP):
        wq_t = np.asarray(Wq[AI * t:AI * (t + 1), :], f32).astype(f16)
        wqs = (wq_t.reshape(4, 512, 4, 8, 128).transpose(0, 2, 4, 3, 1)
               .reshape(-1, 8 * 512))
        per_t[t] = {
            "wqs": np.ascontiguousarray(wqs),
            "wkT": np.ascontiguousarray(
                Wk[G4 * t:G4 * (t + 1), :].T).astype(f16),
            "wvT": np.ascontiguousarray(
                Wv[G4 * t:G4 * (t + 1), :].T).astype(f16),
            "woT": np.ascontiguousarray(
                Wo[AI * t:AI * (t + 1), :].T).astype(f16),
        }
    in_maps = []
    for c in range(N_CORES):
        b, t = c // TP, c % TP
        m = {}
        m.update(per_b[b])
        m.update(per_t[t])
        in_maps.append(m)
    return in_maps


_CACHE = {}


def _get_module(ctx_len=CTX, timing=False):
    key = (ctx_len, timing)
    if key not in _CACHE:
        _CACHE[key] = build(ctx_len, timing=timing)
    return _CACHE[key]


def kernel(hidden_states, target_hidden, cos, sin, Wq, Wk, Wv, Wo,
           q_norm_w, k_norm_w):
    args = [np.asarray(a) for a in (hidden_states, target_hidden, cos, sin,
                                    Wq, Wk, Wv, Wo, q_norm_w, k_norm_w)]
    nc = _get_module(CTX)
    in_maps = host_prep(*args, ctx_len=CTX)
    res = run_bass_kernel_spmd(nc, in_maps, core_ids=list(range(N_CORES)))
    out = np.stack(
        [np.concatenate([res.results[TP * b + t]["y"] for t in range(TP)],
                        axis=1) for b in range(B)], axis=0)
    return out.astype(np.float32)



# revision 25
# speedup vs baseline: 44.8489x; 1.0019x over previous
"""DFlashAttention on 8 Trainium2 NeuronCores.

Sharding: data-parallel over batch (4) x tensor-parallel over heads (2).
Core c = 2*b + t handles batch b, q heads [16t, 16t+16), kv heads [4t, 4t+4).
GQA groups (4 q heads per kv head) align with the head split, so attention is
core-local. Output projection: AllGather of attention outputs over the TP
pair, then each core computes its [128, 2048] output column block.

Fused single-pass pipeline (all matmuls f16, fp32 PSUM):
  The KV projection and attention stream together over 65 chunks of 128 kv
  tokens; K/V never spill to DRAM. Per chunk c: K then V projection (32 hc
  matmuls each, N=512 covering the 4 local kv heads), K rmsnorm+rope on
  scalar/vector (norm weights folded into host-precomputed rope tables),
  PE transpose to kT, then (lagged 8 chunks behind) S^T = kT_g.T @ qT4[g],
  P = exp(S^T) with no max subtraction, O^T += V_g.T @ P accumulated in
  PSUM over chunk pairs and drained to SBUF f32 by the vector engine.
  The softmax denominator l is accumulated elementwise on the vector engine
  (l_acc += P per chunk) and reduced over partitions once at the end with a
  single ones-vector matmul per group - no per-chunk PE work for l.
  Q projection (weights streamed in 8 half-group tiles during chunks 0-7)
  is emitted between chunks 7 and 8.
  Tail: aT = O * (1/l) via ones-outer broadcast matmul, pairwise AllGather,
  out = attnT.T @ WoT with Wo tiles streamed into the dead wku/wvu/xtu
  buffers plus a small tail pool.
"""
import numpy as np

import concourse.mybir as mybir
import concourse.tile as tile
from concourse import bacc
from concourse.bass_utils import run_bass_kernel_spmd
from concourse.masks import make_identity

B, Q_LEN, CTX, HID = 4, 128, 8192, 4096
H, KVH, D = 32, 8, 128
EPS = 1e-6
N_CORES = 8
TP = 2
HL, GL = H // TP, KVH // TP  # 16 local q heads, 4 local kv heads
AI = HL * D                  # 2048 local attention dims
G4 = GL * D                  # 512 = packed 4-head width

F16 = mybir.dt.bfloat16
F32 = mybir.dt.float32
ALU = mybir.AluOpType
ACTF = mybir.ActivationFunctionType

HC = HID // 128              # 32 hidden chunks
LSC = 1.0 / 8192.0           # l rescale so 1/l stays in f16 normal range
LAG = 12                     # S runs LAG chunks behind the KV projection


def build(ctx_len=CTX, timing=False, abl=frozenset()):
    kv_len = ctx_len + Q_LEN
    assert ctx_len % 128 == 0
    n_ctx = ctx_len // 128           # 64 chunks from target_hidden
    n_chunks = kv_len // 128         # 65 total (last chunk = hidden_states)

    nc = bacc.Bacc("TRN2", target_bir_lowering=False, debug=False,
                   num_devices=N_CORES)

    # timing=True turns the big inputs into Internal DRAM scratch (garbage
    # data, identical instruction stream) so per-call input shipping through
    # the axon tunnel vanishes and wall-minus-null resolves HW exec time.
    kb = "Internal" if timing else "ExternalInput"

    # xs/tTs/wqs are host-side pre-arranged into the exact SBUF image
    # ([partition, (hc, col)]) so every DMA line is a multi-KB contiguous run
    xs = nc.dram_tensor("xs", [128, HC * Q_LEN], F16, kind=kb)
    tTs = nc.dram_tensor("tTs", [(ctx_len // 128) * 128, HC * 128], F16,
                         kind=kb)
    wqs = nc.dram_tensor("wqs", [16 * 128, HC * 128], F16,
                         kind=kb)
    wkT = nc.dram_tensor("wkT", [HID, G4], F16, kind=kb)
    wvT = nc.dram_tensor("wvT", [HID, G4], F16, kind=kb)
    woT = nc.dram_tensor("woT", [HID, AI], F16, kind=kb)
    cosq4 = nc.dram_tensor("cosq4", [Q_LEN, 512], F16, kind=kb)
    sinq4 = nc.dram_tensor("sinq4", [Q_LEN, 512], F16, kind=kb)
    cosk4 = nc.dram_tensor("cosk4", [kv_len, 512], F16, kind=kb)
    sink4 = nc.dram_tensor("sink4", [kv_len, 512], F16, kind=kb)
    y = nc.dram_tensor("y", [Q_LEN, AI], F32, kind="ExternalOutput")
    # 1x1 passthrough so chained timing execs have a data dependence
    # (the bass_exec custom_call is CSE-able; see time_hw.py)
    chain_in = nc.dram_tensor("chain_in", [1, 1], F32, kind="ExternalInput")
    chain_out = nc.dram_tensor("chain_out", [1, 1], F32, kind="ExternalOutput")

    eps_ap = [None]

    def rope_block(pool, kfp, src_ap, cos_ap, sin_ap, tag):
        """src [128, 512] f32 (PSUM) -> rms-normed + roped f16 [128, 512].
        cos/sin are f16 [128, 512] with norm weights pre-folded.
        Squares and scaled copies run on the vector engine so the scalar
        engine's activation table (Sqrt/Exp/Copy) stays resident."""
        ksb = pool.tile([128, 512], F16, tag=f"{tag}ksb", name="ksb")
        nc.scalar.copy(ksb[:], src_ap)
        scr = pool.tile([128, 128], F32, tag=f"{tag}scr", name="scr")
        ms4 = pool.tile([128, 4], F32, tag=f"{tag}ms4", name="ms4")
        for g in range(4):
            sl = ksb[:, 128 * g:128 * (g + 1)]
            nc.vector.tensor_mul(scr[:], sl, sl)
            nc.vector.tensor_reduce(ms4[:, g:g + 1], scr[:],
                                    axis=mybir.AxisListType.X, op=ALU.add)
        rms4 = pool.tile([128, 4], F32, tag=f"{tag}rms4", name="rms4")
        nc.scalar.activation(rms4[:], ms4[:], func=ACTF.Sqrt,
                             scale=1.0 / D, bias=eps_ap[0])
        inv4 = pool.tile([128, 4], F32, tag=f"{tag}inv4", name="inv4")
        nc.vector.reciprocal(inv4[:], rms4[:])
        kn = pool.tile([128, 512], F16, tag=f"{tag}kn", name="kn")
        for g in range(4):
            nc.vector.tensor_scalar(kn[:, 128 * g:128 * (g + 1)],
                                    ksb[:, 128 * g:128 * (g + 1)],
                                    inv4[:, g:g + 1], None, op0=ALU.mult)
        knv = kn[:].rearrange("p (g d) -> p g d", d=D)
        sinv = sin_ap.rearrange("p (g d) -> p g d", d=D)
        t1 = pool.tile([128, 512], F16, tag=f"{tag}t1", name="t1")
        nc.vector.tensor_mul(t1[:], kn[:], cos_ap)
        t2 = pool.tile([128, 512], F16, tag=f"{tag}t2", name="t2")
        t2v = t2[:].rearrange("p (g d) -> p g d", d=D)
        nc.vector.scalar_tensor_tensor(
            t2v[:, :, 0:64], knv[:, :, 64:128], -1.0, sinv[:, :, 0:64],
            op0=ALU.mult, op1=ALU.mult)
        nc.vector.tensor_mul(t2v[:, :, 64:128], knv[:, :, 0:64],
                             sinv[:, :, 64:128])
        kf = kfp.tile([128, 512], F16, tag=f"{tag}kf", name="kf")
        nc.vector.tensor_add(kf[:], t1[:], t2[:])
        return kf

    with tile.TileContext(nc) as tc:
        with (
            tc.tile_pool(name="dram", bufs=1, space="DRAM") as dpool,
            tc.tile_pool(name="const", bufs=1) as cpool,
            tc.tile_pool(name="qt", bufs=1) as qtpool,
            tc.tile_pool(name="oacc", bufs=1) as opool,
        ):
            ag_in = dpool.tile([G4, 512], F16)
            ag_out = dpool.tile([TP * G4, 512], F16)

            cq_t = cpool.tile([Q_LEN, 512], F16)
            sq_t = cpool.tile([Q_LEN, 512], F16)
            ident = cpool.tile([128, 128], F16)
            ones = cpool.tile([128, 128], F16)
            eps_t = cpool.tile([128, 1], F32)
            chn_t = cpool.tile([1, 1], F32)
            nc.scalar.dma_start(chn_t[:], chain_in[:, :])
            nc.scalar.dma_start(chain_out[:, :], chn_t[:])

            qT4 = [qtpool.tile([D, 512], F16, name=f"qT4_{g}")
                   for g in range(GL)]
            o_acc = [opool.tile([D, 512], F32, name=f"oacc{g}")
                     for g in range(GL)]
            l_acc = [opool.tile([128, 512], F32, name=f"lacc{g}")
                     for g in range(GL)]

            with tc.tile_pool(name="xw", bufs=1) as xwpool:
                xtu = xwpool.tile([128, HC * Q_LEN], F16, name="xtu")
                wku = xwpool.tile([128, HC * G4], F16, name="wku")
                wvu = xwpool.tile([128, HC * G4], F16, name="wvu")
                xv = xtu[:].rearrange("p (c q) -> p c q", q=Q_LEN)
                wkv_ = wku[:].rearrange("p (c w) -> p c w", w=G4)
                wvv = wvu[:].rearrange("p (c w) -> p c w", w=G4)

                # state threaded between emission helpers
                kfs, kt4s, vsbs, pts = {}, {}, {}, {}
                ops_cur = {}

                with (
                    tc.tile_pool(name="ktb", bufs=LAG + 1) as ktb,
                    tc.tile_pool(name="vb", bufs=LAG + 2) as vbp,
                    tc.tile_pool(name="pt", bufs=9) as ptp,
                    tc.tile_pool(name="rp", bufs=2) as rp,
                    tc.tile_pool(name="kfp", bufs=3) as kfp,
                    tc.tile_pool(name="tt", bufs=2) as ttp,
                    tc.tile_pool(name="cs", bufs=2) as csp,
                    tc.tile_pool(name="kvp", bufs=1, space="PSUM") as kvpp,
                    tc.tile_pool(name="ktp", bufs=1, space="PSUM") as ktpp,
                ):
                    # ---------------- prologue ----------------
                    nc.sync.dma_start(xtu[:], xs[:, :])
                    nc.sync.dma_start(cq_t[:], cosq4[:, :])
                    nc.sync.dma_start(sq_t[:], sinq4[:, :])
                    make_identity(nc, ident[:])
                    nc.vector.memset(ones[:], 1.0)
                    nc.vector.memset(eps_t[:], EPS)
                    eps_ap[0] = eps_t[:]
                    for g in range(GL):
                        nc.vector.memset(o_acc[g][:], 0.0)
                        nc.vector.memset(l_acc[g][:], 0.0)
                    # wk/wv in interleaved pieces so chunk 0 starts early
                    for piece in range(4):
                        hc0, hc1 = 8 * piece, 8 * (piece + 1)
                        nc.sync.dma_start(
                            wkv_[:, hc0:hc1, :],
                            wkT[128 * hc0:128 * hc1, :].rearrange(
                                "(c p) w -> p c w", p=128))
                        nc.sync.dma_start(
                            wvv[:, hc0:hc1, :],
                            wvT[128 * hc0:128 * hc1, :].rearrange(
                                "(c p) w -> p c w", p=128))

                    def emit_kv(c):
                        if c < n_ctx:
                            ttu = ttp.tile([128, HC * 128], F16, tag="tt",
                                           name="ttu")
                            nc.sync.dma_start(
                                ttu[:], tTs[128 * c:128 * (c + 1), :])
                            src = ttu[:].rearrange("p (h w) -> p h w", w=128)
                        else:
                            src = xv
                        ck = csp.tile([128, 512], F16, tag="ck", name="ck")
                        nc.sync.dma_start(
                            ck[:], cosk4[128 * c:128 * (c + 1), :])
                        sk = csp.tile([128, 512], F16, tag="sk", name="sk")
                        nc.sync.dma_start(
                            sk[:], sink4[128 * c:128 * (c + 1), :])
                        kps = kvpp.tile([128, G4], F32, tag="kps", name="kps")
                        for hc in range(HC):
                            nc.tensor.matmul(kps[:], src[:, hc, :],
                                             wkv_[:, hc, :],
                                             start=(hc == 0),
                                             stop=(hc == HC - 1))
                        vps = kvpp.tile([128, G4], F32, tag="vps", name="vps")
                        for hc in range(HC):
                            nc.tensor.matmul(vps[:], src[:, hc, :],
                                             wvv[:, hc, :],
                                             start=(hc == 0),
                                             stop=(hc == HC - 1))
                        if "kv_only" in abl:
                            return
                        if "no_rope" in abl:
                            kf = kfp.tile([128, 512], F16, tag="kkf",
                                          name="kf")
                            nc.scalar.copy(kf[:], kps[:])
                            kfs[c] = kf
                        else:
                            kfs[c] = rope_block(rp, kfp, kps[:], ck[:],
                                                sk[:], "k")
                        vsb = vbp.tile([128, G4], F16, tag="vsb", name="vsb")
                        nc.scalar.copy(vsb[:], vps[:])
                        vsbs[c] = vsb

                    def emit_T(c):
                        ktps = ktpp.tile([D, 512], F16, tag="ktps",
                                         name="ktps")
                        kf = kfs.pop(c)
                        for g in range(4):
                            nc.tensor.transpose(
                                ktps[:, 128 * g:128 * (g + 1)],
                                kf[:, 128 * g:128 * (g + 1)], ident[:])
                        kt4 = ktb.tile([D, 512], F16, tag="kt4", name="kt4")
                        nc.scalar.copy(kt4[:], ktps[:])
                        kt4s[c] = kt4

                    def emit_S(c, gs):
                        for g in gs:
                            sp = stp.tile([128, 512], F32, tag="stp",
                                          name="sp")
                            nc.tensor.matmul(
                                sp[:], kt4s[c][:, 128 * g:128 * (g + 1)],
                                qT4[g][:], start=True, stop=True)
                            pt = ptp.tile([128, 512], F16, tag="pt",
                                          name="pt")
                            nc.scalar.activation(pt[:], sp[:], func=ACTF.Exp)
                            nc.vector.tensor_add(l_acc[g][:], l_acc[g][:],
                                                 pt[:])
                            pts[(c, g)] = pt
                        if gs[-1] == GL - 1:
                            kt4s.pop(c)

                    def emit_O(c, gs):
                        if c < 0:
                            return
                        start = (c % 4 == 0)
                        stop = (c % 4 == 3) or (c == n_chunks - 1)
                        for g in gs:
                            if start:
                                ops_cur[g] = opp.tile([D, 512], F32,
                                                      tag="ops", name="ops")
                            nc.tensor.matmul(
                                ops_cur[g][:],
                                vsbs[c][:, 128 * g:128 * (g + 1)],
                                pts.pop((c, g))[:], start=start, stop=stop)
                            if stop:
                                nc.vector.tensor_add(o_acc[g][:],
                                                     o_acc[g][:],
                                                     ops_cur[g][:])
                        if gs[-1] == GL - 1:
                            vsbs.pop(c)

                    # ---------------- warmup chunks + Q ----------------
                    # Wq streams as 16 quarter-group tiles on the otherwise
                    # idle gpsimd DMA queue (ring waits there cannot block
                    # the tt/cos/sin stream on the sync queue). Q-projection
                    # groups are interleaved after warmup chunks 4..7 so the
                    # ring drains while the PE works.
                    wq_tiles = {}
                    with (
                        tc.tile_pool(name="wq", bufs=4) as wqp,
                        tc.tile_pool(name="qps", bufs=2, space="PSUM") as qpp,
                        tc.tile_pool(name="qtp", bufs=2, space="PSUM") as qtp,
                    ):
                        def load_wq(i):
                            t = wqp.tile([128, HC * 128], F16, tag="wq",
                                         name="wqh")
                            nc.gpsimd.dma_start(
                                t[:], wqs[128 * i:128 * (i + 1), :])
                            wq_tiles[i] = t

                        def emit_Q(g):
                            # wq tiles are hc-major: tile (4g+i) holds hc
                            # range [8i, 8i+8) x the full 512 group width,
                            # so every Q matmul runs at N=512
                            qps = qpp.tile([Q_LEN, 512], F32, tag="qps",
                                           name="qps")
                            for i in range(4):
                                wqv = wq_tiles[4 * g + i][:].rearrange(
                                    "p (c w) -> p c w", w=512)
                                for hl in range(8):
                                    hc = 8 * i + hl
                                    nc.tensor.matmul(
                                        qps[:], xv[:, hc, :],
                                        wqv[:, hl, :],
                                        start=(hc == 0),
                                        stop=(hc == HC - 1))
                            qf = rope_block(rp, kfp, qps[:], cq_t[:],
                                            sq_t[:], "q")
                            qtps = qtp.tile([D, 512], F16, tag="qtps",
                                            name="qtps")
                            for hh in range(4):
                                nc.tensor.transpose(
                                    qtps[:, 128 * hh:128 * (hh + 1)],
                                    qf[:, 128 * hh:128 * (hh + 1)],
                                    ident[:])
                            nc.scalar.copy(qT4[g][:], qtps[:])

                        wq_sched = {0: [0, 1, 2, 3]}
                        for c in range(1, 12):
                            wq_sched[c] = [c + 3] + ([15] if c == 11 else [])
                        for c in range(LAG):
                            emit_kv(c)
                            for i in wq_sched.get(c, []):
                                load_wq(i)
                            if c >= 1:
                                emit_T(c - 1)
                            if c >= LAG - 4:
                                emit_Q(c - (LAG - 4))

                    # ---------------- fused steady loop ----------------
                    with (
                        tc.tile_pool(name="stp", bufs=3,
                                     space="PSUM") as stp,
                        tc.tile_pool(name="opp", bufs=2,
                                     space="PSUM") as opp,
                    ):
                        for c in range(LAG, n_chunks):
                            emit_kv(c)
                            emit_T(c - 1)
                            emit_S(c - LAG, (0, 1))
                            emit_O(c - LAG - 1, (0, 1))
                            emit_S(c - LAG, (2, 3))
                            emit_O(c - LAG - 1, (2, 3))
                        # catchup: attention for the last LAG chunks
                        emit_T(n_chunks - 1)
                        for c in range(n_chunks - LAG, n_chunks):
                            emit_S(c, (0, 1))
                            emit_O(c - 1, (0, 1))
                            emit_S(c, (2, 3))
                            emit_O(c - 1, (2, 3))
                        emit_O(n_chunks - 1, (0, 1))
                        emit_O(n_chunks - 1, (2, 3))

                # ---------------- tail: scale, allgather, Wo ----------
                # Wo tiles stream into the dead wku/wvu/xtu buffers
                # (8 + 8 + 2 head-blocks) plus a 14-tile tail pool.
                NH = TP * HL
                NJ = AI // 512
                slot_views = []
                for t in (wku, wvu, xtu):
                    v = t[:].rearrange("p (s w) -> p s w", w=AI)
                    for s in range(v.shape[1]):
                        slot_views.append(v[:, s, :])
                with (
                    tc.tile_pool(name="wot", bufs=14) as wotp,
                    tc.tile_pool(name="tsb", bufs=2) as tsb,
                    tc.tile_pool(name="agp", bufs=1) as agp,
                    tc.tile_pool(name="wps", bufs=1, space="PSUM") as wopp,
                    tc.tile_pool(name="lsp", bufs=1, space="PSUM") as lsp,
                    tc.tile_pool(name="bcps", bufs=2, space="PSUM") as bcpp,
                ):
                    wo_aps = {}

                    def load_wo(h, eng):
                        if h < 18:
                            dst = slot_views[h]
                        else:
                            t = wotp.tile([128, AI], F16, tag="wo",
                                          name="wot")
                            dst = t[:]
                        eng.dma_start(dst, woT[128 * h:128 * (h + 1), :])
                        wo_aps[h] = dst

                    # slot loads reuse the dead wku/wvu/xtu buffers; they
                    # stream on the gpsimd queue during the catchup chunks
                    for h in range(18):
                        load_wo(h, nc.gpsimd)

                    # broadcast l across partitions first (ones-outer
                    # matmul), then reciprocal on the full [128, 512] tile -
                    # a [1, 512] reciprocal would run on a single DVE lane
                    # stage-interleaved so the four groups' chains pipeline
                    # across scalar/PE/vector instead of running serially
                    aT4 = [tsb.tile([D, 512], F16, tag=f"aT4_{g}",
                                    name=f"aT4_{g}") for g in range(GL)]
                    l16s, lsbs = [], []
                    for g in range(GL):
                        l16 = tsb.tile([128, 512], F16, tag=f"l16_{g}",
                                       name="l16")
                        nc.scalar.copy(l16[:], l_acc[g][:])
                        l16s.append(l16)
                    for g in range(GL):
                        lsum = lsp.tile([1, 512], F32, tag="lsum",
                                        name="lsum")
                        nc.tensor.matmul(lsum[:], ones[:, 0:1], l16s[g][:],
                                         start=True, stop=True)
                        lsb = tsb.tile([1, 512], F16, tag=f"lsb_{g}",
                                       name="lsb")
                        nc.scalar.copy(lsb[:], lsum[:])
                        lsbs.append(lsb)
                    for g in range(GL):
                        bcl = bcpp.tile([128, 512], F32, tag="bcl",
                                        name="bcl")
                        for hh in range(4):
                            cs = slice(128 * hh, 128 * (hh + 1))
                            nc.tensor.matmul(bcl[:, cs], ones[0:1, :],
                                             lsbs[g][:, cs], start=True,
                                             stop=True)
                        invb = tsb.tile([128, 512], F32, tag="invb",
                                        name="invb")
                        nc.vector.reciprocal(invb[:], bcl[:])
                        nc.vector.scalar_tensor_tensor(
                            aT4[g][:], o_acc[g][:], 1.0, invb[:],
                            op0=ALU.mult, op1=ALU.mult)
                        nc.sync.dma_start(
                            ag_in[128 * g:128 * (g + 1), :], aT4[g][:])
                    nc.gpsimd.collective_compute(
                        "AllGather",
                        ALU.bypass,
                        replica_groups=[[0, 1], [2, 3], [4, 5], [6, 7]],
                        ins=[ag_in.opt()],
                        outs=[ag_out.opt()],
                    )
                    # remaining Wo tiles stream on the scalar queue (idle
                    # in the tail) so the ag DMAs on sync aren't blocked
                    for h in range(18, NH):
                        load_wo(h, nc.scalar)
                    ag_sb = []
                    for t in range(TP):
                        a_sb = agp.tile([128, GL * 512], F16,
                                        name=f"ag_sb{t}")
                        nc.sync.dma_start(
                            a_sb[:].rearrange("p (g w) -> p g w", w=512),
                            ag_out[G4 * t:G4 * (t + 1), :].rearrange(
                                "(g p) w -> p g w", p=128))
                        ag_sb.append(a_sb)
                    wps = [wopp.tile([Q_LEN, 512], F32, tag=f"wps{j}",
                                     name="wps") for j in range(NJ)]
                    for h in range(NH):
                        t, g, hh = h // 16, (h % 16) // 4, h % 4
                        lhs = ag_sb[t][:].rearrange("p (g w) -> p g w",
                                                    w=512)
                        for j in range(NJ):
                            nc.tensor.matmul(
                                wps[j][:],
                                lhs[:, g, 128 * hh:128 * (hh + 1)],
                                wo_aps[h][:, 512 * j:512 * (j + 1)],
                                start=(h == 0), stop=(h == NH - 1))
                    for j in range(NJ):
                        out_j = tsb.tile([Q_LEN, 512], F32, tag="outj",
                                         name="outj")
                        nc.vector.tensor_copy(out_j[:], wps[j][:])
                        nc.sync.dma_start(y[:, 512 * j:512 * (j + 1)],
                                          out_j[:])

    nc.compile()
    return nc


def host_prep(hidden_states, target_hidden, cos, sin, Wq, Wk, Wv, Wo,
              q_norm_w, k_norm_w, ctx_len=CTX):
    """Build the 8 per-core input maps from full inputs (numpy, host side)."""
    kv_len = ctx_len + Q_LEN
    f16 = mybir.dt.np(mybir.dt.bfloat16)
    f32 = np.float32

    qw = np.asarray(q_norm_w, f32)
    kw = np.asarray(k_norm_w, f32)
    qw_rot = np.concatenate([qw[64:], qw[:64]])
    kw_rot = np.concatenate([kw[64:], kw[:64]])
    scale = np.float32(D ** -0.5)

    per_b = {}
    for b in range(B):
        cq = np.asarray(cos[b, ctx_len:kv_len], f32)
        sq = np.asarray(sin[b, ctx_len:kv_len], f32)
        ckv = np.asarray(cos[b, :kv_len], f32)
        skv = np.asarray(sin[b, :kv_len], f32)
        xs = (np.asarray(hidden_states[b], f32).T.astype(f16)
              .reshape(HC, 128, Q_LEN).transpose(1, 0, 2).reshape(128, -1))
        tts = (np.asarray(target_hidden[b], f32).astype(f16)
               .reshape(ctx_len // 128, 128, HC, 128)
               .transpose(0, 3, 2, 1).reshape(-1, HC * 128))
        per_b[b] = {
            "chain_in": np.zeros((1, 1), np.float32),
            "xs": np.ascontiguousarray(xs),
            "tTs": np.ascontiguousarray(tts),
            "cosq4": np.ascontiguousarray(
                np.tile((cq * qw * scale).astype(f16), (1, 4))),
            "sinq4": np.ascontiguousarray(
                np.tile((sq * qw_rot * scale).astype(f16), (1, 4))),
            "cosk4": np.ascontiguousarray(
                np.tile((ckv * kw).astype(f16), (1, 4))),
            "sink4": np.ascontiguousarray(
                np.tile((skv * kw_rot).astype(f16), (1, 4))),
        }
    per_t = {}
    for t in range(TP):
        wq_t = np.asarray(Wq[AI * t:AI * (t + 1), :], f32).astype(f16)
        wqs = (wq_t.reshape(4, 512, 4, 8, 128).transpose(0, 2, 4, 3, 1)
               .reshape(-1, 8 * 512))
        per_t[t] = {
            "wqs": np.ascontiguousarray(wqs),
            "wkT": np.ascontiguousarray(
                Wk[G4 * t:G4 * (t + 1), :].T).astype(f16),
            "wvT": np.ascontiguousarray(
                Wv[G4 * t:G4 * (t + 1), :].T).astype(f16),
            "woT": np.ascontiguousarray(
                Wo[AI * t:AI * (t + 1), :].T).astype(f16),
        }
    in_maps = []
    for c in range(N_CORES):
        b, t = c // TP, c % TP
        m = {}
        m.update(per_b[b])
        m.update(per_t[t])
        in_maps.append(m)
    return in_maps


_CACHE = {}


def _get_module(ctx_len=CTX, timing=False):
    key = (ctx_len, timing)
    if key not in _CACHE:
        _CACHE[key] = build(ctx_len, timing=timing)
    return _CACHE[key]


def kernel(hidden_states, target_hidden, cos, sin, Wq, Wk, Wv, Wo,
           q_norm_w, k_norm_w):
    args = [np.asarray(a) for a in (hidden_states, target_hidden, cos, sin,
                                    Wq, Wk, Wv, Wo, q_norm_w, k_norm_w)]
    nc = _get_module(CTX)
    in_maps = host_prep(*args, ctx_len=CTX)
    res = run_bass_kernel_spmd(nc, in_maps, core_ids=list(range(N_CORES)))
    out = np.stack(
        [np.concatenate([res.results[TP * b + t]["y"] for t in range(TP)],
                        axis=1) for b in range(B)], axis=0)
    return out.astype(np.float32)

